# revision 24
# baseline (speedup 1.0000x reference)
"""nn_AttenDense Trainium2 Bass kernel.

Computation (reference):
    xf = x.reshape(-1, D)                       # [N, D]
    scores = xf @ atten_emb.T                   # [N, C]
    probs = softmax(scores, axis=-1)
    xr = xf + probs @ atten_emb                 # [N, D]
    y = leaky_relu(xr @ W.T + b, 0.01)          # [N, C]

Strategy: data-parallel over the N=8192 flattened tokens across 8 cores
(1024 tokens each); atten_emb / W / b replicated. Per core everything is
computed in "transposed" layout (tokens on the free axis) so no on-chip
transposes are ever needed:

  mm1 (f32r): scoresT[c, n] = AT_tile.T @ xT        (contract d)
  exp (ACT):  expT = exp(scoresT)  -> bf16, resident in SBUF
  den (DVE+GPSIMD): den[n] = sum_c expT  -> partition_all_reduce -> 1/den
  mm2 (bf16): attnT[d, n] = A_tile.T @ expT         (contract c)
  res (DVE):  xrT = xT + attnT * recip[n]   (in-place over xT)
  mm3 (f32r): yT[c, n] = WT_tile.T @ xrT            (contract d)
  act (ACT):  y = Lrelu(yT + b[c])   (bias per-partition, alpha=0.01)

Softmax is computed without max-subtraction: |scores| <= ~25 for any
plausible data here (exp fits fp32/bf16 range with huge margin), and
exp(s)/sum(exp(s)) is algebraically identical to the max-subtracted form.

All DRAM inputs are pre-tiled on the host so every DMA is a single fully
contiguous block.
"""

import numpy as np
import ml_dtypes

NEG_SLOPE = 0.01
P = 128
FREE = 512
NCORES = 8


def build_bass(nloc, d, c, debug=False):
    """Build the per-core Bass program (SPMD: same NEFF, per-core data)."""
    import concourse.bass as bass
    import concourse.bacc as bacc
    import concourse.mybir as mybir
    from concourse.tile import TileContext
    from concourse.bass_isa import ReduceOp

    f32 = mybir.dt.float32
    f32r = mybir.dt.float32r
    bf16 = mybir.dt.bfloat16
    Exp = mybir.ActivationFunctionType.Exp
    Identity = mybir.ActivationFunctionType.Identity

    kd = d // P          # d-tiles (contraction tiles for mm1/mm3)
    ct = c // P          # c-tiles
    free = min(FREE, nloc)
    nh = nloc // free    # n-halves per row of 512

    nc = bacc.Bacc("TRN2", target_bir_lowering=False, debug=debug)

    # float32r = fp32 bits matmul'd at 1 cycle/row (vs 4 for fp32). The BIR
    # verifier requires the whole def-chain feeding an fp32r matmul to carry
    # the f32r dtype, so these are declared f32r from DRAM onward.
    xt = nc.dram_tensor("xt", [P, kd, nloc], f32r, kind="ExternalInput")
    xbf = nc.dram_tensor("xbf", [P, kd, nloc], bf16, kind="ExternalInput")
    at = nc.dram_tensor("at", [ct, P, kd, P], bf16, kind="ExternalInput")
    abf = nc.dram_tensor("abf", [kd, P, ct, P], bf16, kind="ExternalInput")
    wt = nc.dram_tensor("wt", [ct, P, kd, P], f32r, kind="ExternalInput")
    bias = nc.dram_tensor("bias", [P, ct], f32, kind="ExternalInput")
    yt = nc.dram_tensor("yt", [ct, P, nloc], f32, kind="ExternalOutput")

    with TileContext(nc) as tc:
        with (
            tc.tile_pool(name="persist", bufs=1) as persist,
            tc.tile_pool(name="lhs", bufs=4) as lhs_pool,
            tc.tile_pool(name="tmp", bufs=3) as tmp_pool,
            tc.tile_pool(name="ystage", bufs=3) as ystage,
            tc.tile_pool(name="psA", bufs=4, space="PSUM") as psA,
            tc.tile_pool(name="psB", bufs=4, space="PSUM") as psB,
        ):
            # ---- persistent tiles ----
            # weight streams ride the SP HWDGE ring (nc.sync); x / bias / y
            # ride the ACT ring (nc.scalar) so neither ring head-blocks the
            # other. xt is split per k-tile so the first matmul can start
            # after ~1/8 of the load.
            xbf_sb = persist.tile([P, kd, nloc], bf16, tag="xbf_sb")
            for k in range(kd):
                nc.scalar.dma_start(out=xbf_sb[:, k], in_=xbf[:, k])
            # f32r copy of x (residual + mm3 rhs) — not needed until the end
            # of phase B, so it loads behind the bf16 copy
            xt_sb = persist.tile([P, kd, nloc], f32r, tag="xt_sb")
            for k in range(kd):
                nc.scalar.dma_start(out=xt_sb[:, k], in_=xt[:, k])
            bias_sb = persist.tile([P, ct], f32, tag="bias_sb")
            nc.scalar.dma_start(out=bias_sb[:], in_=bias[:])

            # PE warmup: dummy matmuls with no DMA dependency keep the PE
            # busy through the HAM activity window during the initial x/A
            # load, so real matmuls start at the full 2.4 GHz clock.
            warm_sb = persist.tile([P, free], bf16, tag="warm_sb")
            nc.gpsimd.memset(warm_sb[:], 0.0)
            warm_ps = psB.tile([P, free], f32, tag="psB", name="warm_ps")
            for _ in range(10):
                nc.tensor.matmul(
                    warm_ps[:], lhsT=warm_sb[:, :P], rhs=warm_sb[:]
                )
            # keep a reader so the warmup chain isn't dead-code-eliminated
            nc.vector.tensor_copy(out=warm_sb[:], in_=warm_ps[:])

            expT = [
                persist.tile([P, nloc], bf16, tag=f"expT{i}", name=f"expT{i}")
                for i in range(ct)
            ]
            acc = persist.tile([P, nloc], f32, tag="acc")
            den = persist.tile([P, nloc], f32, tag="den")
            rec = persist.tile([P, nloc], f32, tag="rec")

            # ---- phase A: scoresT = AT.T @ xT ; expT = exp(scoresT) ----
            for i in range(ct):
                at_sb = lhs_pool.tile([P, kd, P], bf16, tag="at")
                nc.sync.dma_start(out=at_sb[:], in_=at[i])
                # k-outer / h-inner: each weight tile is loaded once and used
                # for both n-halves (halves LDWEIGHTS traffic)
                ps = [
                    psA.tile([P, free], f32, tag="psA", name=f"psA_{i}_{h}")
                    for h in range(nh)
                ]
                for k in range(kd):
                    for h in range(nh):
                        sl = slice(h * free, (h + 1) * free)
                        nc.tensor.matmul(
                            ps[h][:],
                            lhsT=at_sb[:, k],
                            rhs=xbf_sb[:, k, sl],
                            start=(k == 0),
                            stop=(k == kd - 1),
                        )
                for h in range(nh):
                    sl = slice(h * free, (h + 1) * free)
                    nc.scalar.activation(expT[i][:, sl], ps[h][:], Exp)
                # softmax denominator: running per-partition sum over c-tiles
                if i == 0:
                    nc.vector.tensor_copy(out=acc[:], in_=expT[0][:])
                else:
                    nc.vector.tensor_add(out=acc[:], in0=acc[:], in1=expT[i][:])

            # ---- denominator: sum over partitions (broadcast), reciprocal ----
            nc.gpsimd.partition_all_reduce(den[:], acc[:], 128, ReduceOp.add)
            nc.vector.reciprocal(rec[:], den[:])

            # ---- phase B: attnT = A.T @ expT ; xr = xT + attnT * rec ----
            for j in range(kd):
                a_sb = lhs_pool.tile([P, ct, P], bf16, tag="abf")
                nc.sync.dma_start(out=a_sb[:], in_=abf[j])
                ps = [
                    psB.tile([P, free], f32, tag="psB", name=f"psB_{j}_{h}")
                    for h in range(nh)
                ]
                for i2 in range(ct):
                    for h in range(nh):
                        sl = slice(h * free, (h + 1) * free)
                        nc.tensor.matmul(
                            ps[h][:],
                            lhsT=a_sb[:, i2],
                            rhs=expT[i2][:, sl],
                            start=(i2 == 0),
                            stop=(i2 == ct - 1),
                        )
                for h in range(nh):
                    sl = slice(h * free, (h + 1) * free)
                    tmp = tmp_pool.tile([P, free], f32, tag="tmp")
                    nc.vector.tensor_mul(out=tmp[:], in0=ps[h][:], in1=rec[:, sl])
                    nc.vector.tensor_add(
                        out=xt_sb[:, j, sl], in0=xt_sb[:, j, sl], in1=tmp[:]
                    )

            # ---- phase C: yT = WT.T @ xrT ; y = Lrelu(yT + b) ----
            for i in range(ct):
                wt_sb = lhs_pool.tile([P, kd, P], f32r, tag="wt")
                nc.sync.dma_start(out=wt_sb[:], in_=wt[i])
                yst = ystage.tile([P, nloc], f32, tag="yst")
                # reuse the psA pool slots (phase A tiles are drained by now)
                ps = [
                    psA.tile([P, free], f32, tag="psA", name=f"psC_{i}_{h}")
                    for h in range(nh)
                ]
                for k in range(kd):
                    for h in range(nh):
                        sl = slice(h * free, (h + 1) * free)
                        nc.tensor.matmul(
                            ps[h][:],
                            lhsT=wt_sb[:, k],
                            rhs=xt_sb[:, k, sl],
                            start=(k == 0),
                            stop=(k == kd - 1),
                        )
                for h in range(nh):
                    sl = slice(h * free, (h + 1) * free)
                    # yb = psum + bias (ACT, per-partition bias), then
                    # leaky = max(0.01*yb, yb) in one DVE op
                    nc.scalar.activation(
                        yst[:, sl],
                        ps[h][:],
                        Identity,
                        bias=bias_sb[:, i : i + 1],
                    )
                    nc.vector.scalar_tensor_tensor(
                        out=yst[:, sl],
                        in0=yst[:, sl],
                        scalar=NEG_SLOPE,
                        in1=yst[:, sl],
                        op0=mybir.AluOpType.mult,
                        op1=mybir.AluOpType.max,
                    )
                nc.sync.dma_start(out=yt[i], in_=yst[:])

    nc.compile()
    return nc


def prep_inputs(x, atten_emb, W, b, ncores=NCORES):
    """Host-side shard + pre-tile. Returns (in_maps, dims)."""
    x = np.asarray(x, dtype=np.float32)
    A = np.asarray(atten_emb, dtype=np.float32)
    Wf = np.asarray(W, dtype=np.float32)
    bf = np.asarray(b, dtype=np.float32)

    D = x.shape[-1]
    C = A.shape[0]
    N = int(np.prod(x.shape[:-1]))
    nloc = N // ncores
    kd, ct = D // P, C // P

    xf = np.ascontiguousarray(x.reshape(N, D))

    # at[i, p, k, cc] = A[i*P+cc, k*P+p]   (lhsT for mm1: partition=d, free=c)
    at_t = np.ascontiguousarray(
        A.astype(ml_dtypes.bfloat16).reshape(ct, P, kd, P).transpose(0, 3, 2, 1)
    )
    # abf[j, p, i, dd] = A_bf16[i*P+p, j*P+dd]  (lhsT for mm2: partition=c, free=d)
    a_bf = np.ascontiguousarray(
        A.astype(ml_dtypes.bfloat16).reshape(ct, P, kd, P).transpose(2, 1, 0, 3)
    )
    # wt[i, p, k, cc] = W[i*P+cc, k*P+p]   (lhsT for mm3)
    wt_t = np.ascontiguousarray(Wf.reshape(ct, P, kd, P).transpose(0, 3, 2, 1))
    # bias[p, i] = b[i*P+p]
    bias_t = np.ascontiguousarray(bf.reshape(ct, P).T)

    in_maps = []
    for core in range(ncores):
        xl = xf[core * nloc : (core + 1) * nloc]  # [nloc, D]
        # xt[p, k, n] = xl[n, k*P+p]
        xt = np.ascontiguousarray(xl.T.reshape(kd, P, nloc).transpose(1, 0, 2))
        in_maps.append(
            {
                "xt": xt,
                "xbf": xt.astype(ml_dtypes.bfloat16),
                "at": at_t,
                "abf": a_bf,
                "wt": wt_t,
                "bias": bias_t,
            }
        )
    return in_maps, (N, D, C, nloc)


def assemble_output(results, shape_in, C, nloc):
    """Gather per-core yt [ct, P, nloc] -> full y with original leading dims."""
    N = int(np.prod(shape_in[:-1]))
    y = np.empty((N, C), dtype=np.float32)
    for core, r in enumerate(results):
        yt = r["yt"]  # [ct, P, nloc]; yt[i, p, n] = y_core[n, i*P+p]
        y[core * nloc : (core + 1) * nloc] = yt.reshape(C, nloc).T
    return y.reshape(shape_in[:-1] + (C,))


_CACHED = {}


def _get_nc(nloc, d, c):
    key = (nloc, d, c)
    if key not in _CACHED:
        _CACHED[key] = build_bass(nloc, d, c)
    return _CACHED[key]


def kernel(x, atten_emb, W, b, trace=False):
    from concourse.bass_utils import run_bass_kernel_spmd

    in_maps, (N, D, C, nloc) = prep_inputs(x, atten_emb, W, b)
    nc = _get_nc(nloc, D, C)
    res = run_bass_kernel_spmd(
        nc, in_maps, core_ids=list(range(NCORES)), trace=trace
    )
    y = assemble_output(res.results, tuple(x.shape), C, nloc)
    if trace:
        kernel.last_result = res
    return y


# revision 26
# speedup vs baseline: 1.2220x; 1.2220x over previous
"""nn_AttenDense Trainium2 Bass kernel.

Computation (reference):
    xf = x.reshape(-1, D)                       # [N, D]
    scores = xf @ atten_emb.T                   # [N, C]
    probs = softmax(scores, axis=-1)
    xr = xf + probs @ atten_emb                 # [N, D]
    y = leaky_relu(xr @ W.T + b, 0.01)          # [N, C]

Strategy: data-parallel over the N=8192 flattened tokens across 8 cores
(1024 tokens each); atten_emb / W / b replicated. Per core everything is
computed in "transposed" layout (tokens on the free axis) so no on-chip
transposes are ever needed:

  mm1 (f32r): scoresT[c, n] = AT_tile.T @ xT        (contract d)
  exp (ACT):  expT = exp(scoresT)  -> bf16, resident in SBUF
  den (DVE+GPSIMD): den[n] = sum_c expT  -> partition_all_reduce -> 1/den
  mm2 (bf16): attnT[d, n] = A_tile.T @ expT         (contract c)
  res (DVE):  xrT = xT + attnT * recip[n]   (in-place over xT)
  mm3 (f32r): yT[c, n] = WT_tile.T @ xrT            (contract d)
  act (ACT):  y = Lrelu(yT + b[c])   (bias per-partition, alpha=0.01)

Softmax is computed without max-subtraction: |scores| <= ~25 for any
plausible data here (exp fits fp32/bf16 range with huge margin), and
exp(s)/sum(exp(s)) is algebraically identical to the max-subtracted form.

All DRAM inputs are pre-tiled on the host so every DMA is a single fully
contiguous block.
"""

import numpy as np
import ml_dtypes

NEG_SLOPE = 0.01
P = 128
FREE = 512
NCORES = 8


def build_bass(nloc, d, c, debug=False):
    """Build the per-core Bass program (SPMD: same NEFF, per-core data)."""
    import concourse.bass as bass
    import concourse.bacc as bacc
    import concourse.mybir as mybir
    from concourse.tile import TileContext
    from concourse.bass_isa import ReduceOp

    f32 = mybir.dt.float32
    f32r = mybir.dt.float32r
    bf16 = mybir.dt.bfloat16
    Exp = mybir.ActivationFunctionType.Exp
    Identity = mybir.ActivationFunctionType.Identity

    kd = d // P          # d-tiles (contraction tiles for mm1/mm3)
    ct = c // P          # c-tiles
    free = min(FREE, nloc)
    nh = nloc // free    # n-halves per row of 512

    nc = bacc.Bacc("TRN2", target_bir_lowering=False, debug=debug)

    # float32r = fp32 bits matmul'd at 1 cycle/row (vs 4 for fp32). The BIR
    # verifier requires the whole def-chain feeding an fp32r matmul to carry
    # the f32r dtype, so these are declared f32r from DRAM onward.
    xt = nc.dram_tensor("xt", [P, kd, nloc], f32r, kind="ExternalInput")
    xbf = nc.dram_tensor("xbf", [P, kd, nloc], bf16, kind="ExternalInput")
    at = nc.dram_tensor("at", [ct, P, kd, P], bf16, kind="ExternalInput")
    abf = nc.dram_tensor("abf", [kd, P, ct, P], bf16, kind="ExternalInput")
    wt = nc.dram_tensor("wt", [ct, P, kd, P], f32r, kind="ExternalInput")
    bias = nc.dram_tensor("bias", [P, ct], f32, kind="ExternalInput")
    yt = nc.dram_tensor("yt", [ct, P, nloc], f32, kind="ExternalOutput")

    with TileContext(nc) as tc:
        with (
            tc.tile_pool(name="persist", bufs=1) as persist,
            tc.tile_pool(name="lhs", bufs=4) as lhs_pool,
            tc.tile_pool(name="tmp", bufs=3) as tmp_pool,
            tc.tile_pool(name="ystage", bufs=3) as ystage,
            tc.tile_pool(name="psA", bufs=4, space="PSUM") as psA,
            tc.tile_pool(name="psB", bufs=4, space="PSUM") as psB,
        ):
            # ---- persistent tiles ----
            # weight streams ride the SP HWDGE ring (nc.sync); x / bias / y
            # ride the ACT ring (nc.scalar) so neither ring head-blocks the
            # other. xt is split per k-tile so the first matmul can start
            # after ~1/8 of the load.
            xbf_sb = persist.tile([P, kd, nloc], bf16, tag="xbf_sb")
            for k in range(kd):
                nc.scalar.dma_start(out=xbf_sb[:, k], in_=xbf[:, k])
            # f32r copy of x (residual + mm3 rhs) — not needed until phase
            # B, so its DMAs are interleaved into the SP ring mid-phase-A
            # (emitted inside the phase-A loop below)
            xt_sb = persist.tile([P, kd, nloc], f32r, tag="xt_sb")
            bias_sb = persist.tile([P, ct], f32, tag="bias_sb")
            nc.scalar.dma_start(out=bias_sb[:], in_=bias[:])

            # PE warmup: dummy matmuls with no DMA dependency keep the PE
            # busy through the HAM activity window during the initial x/A
            # load, so real matmuls start at the full 2.4 GHz clock.
            warm_sb = persist.tile([P, free], bf16, tag="warm_sb")
            nc.gpsimd.memset(warm_sb[:], 0.0)
            warm_ps = psB.tile([P, free], f32, tag="psB", name="warm_ps")
            for _ in range(10):
                nc.tensor.matmul(
                    warm_ps[:], lhsT=warm_sb[:, :P], rhs=warm_sb[:]
                )
            # keep a reader so the warmup chain isn't dead-code-eliminated
            nc.vector.tensor_copy(out=warm_sb[:], in_=warm_ps[:])

            expT = [
                persist.tile([P, nloc], bf16, tag=f"expT{i}", name=f"expT{i}")
                for i in range(ct)
            ]
            acc = persist.tile([P, nloc], f32, tag="acc")
            den = persist.tile([P, nloc], f32, tag="den")
            rec = persist.tile([P, nloc], f32, tag="rec")

            # ---- phase A: scoresT = AT.T @ xT ; expT = exp(scoresT) ----
            for i in range(ct):
                at_sb = lhs_pool.tile([P, kd, P], bf16, tag="at")
                nc.sync.dma_start(out=at_sb[:], in_=at[i])
                xt0 = min(8, ct - kd)  # stream the f32r x copy mid-phase-A
                if xt0 <= i < xt0 + kd:
                    nc.sync.dma_start(out=xt_sb[:, i - xt0], in_=xt[:, i - xt0])
                # k-outer / h-inner: each weight tile is loaded once and used
                # for both n-halves (halves LDWEIGHTS traffic)
                ps = [
                    psA.tile([P, free], f32, tag="psA", name=f"psA_{i}_{h}")
                    for h in range(nh)
                ]
                for k in range(kd):
                    for h in range(nh):
                        sl = slice(h * free, (h + 1) * free)
                        nc.tensor.matmul(
                            ps[h][:],
                            lhsT=at_sb[:, k],
                            rhs=xbf_sb[:, k, sl],
                            start=(k == 0),
                            stop=(k == kd - 1),
                        )
                for h in range(nh):
                    sl = slice(h * free, (h + 1) * free)
                    nc.scalar.activation(expT[i][:, sl], ps[h][:], Exp)
                # softmax denominator: running per-partition sum over c-tiles
                if i == 0:
                    nc.vector.tensor_copy(out=acc[:], in_=expT[0][:])
                else:
                    nc.vector.tensor_add(out=acc[:], in0=acc[:], in1=expT[i][:])

            # ---- denominator: sum over partitions (broadcast), reciprocal ----
            nc.gpsimd.partition_all_reduce(den[:], acc[:], 128, ReduceOp.add)
            nc.vector.reciprocal(rec[:], den[:])

            # ---- phase B: attnT = A.T @ expT ; xr = xT + attnT * rec ----
            for j in range(kd):
                a_sb = lhs_pool.tile([P, ct, P], bf16, tag="abf")
                nc.sync.dma_start(out=a_sb[:], in_=abf[j])
                ps = [
                    psB.tile([P, free], f32, tag="psB", name=f"psB_{j}_{h}")
                    for h in range(nh)
                ]
                for i2 in range(ct):
                    for h in range(nh):
                        sl = slice(h * free, (h + 1) * free)
                        nc.tensor.matmul(
                            ps[h][:],
                            lhsT=a_sb[:, i2],
                            rhs=expT[i2][:, sl],
                            start=(i2 == 0),
                            stop=(i2 == ct - 1),
                        )
                for h in range(nh):
                    sl = slice(h * free, (h + 1) * free)
                    tmp = tmp_pool.tile([P, free], f32, tag="tmp")
                    nc.vector.tensor_mul(out=tmp[:], in0=ps[h][:], in1=rec[:, sl])
                    nc.vector.tensor_add(
                        out=xt_sb[:, j, sl], in0=xt_sb[:, j, sl], in1=tmp[:]
                    )

            # ---- phase C: yT = WT.T @ xrT ; y = Lrelu(yT + b) ----
            for i in range(ct):
                wt_sb = lhs_pool.tile([P, kd, P], f32r, tag="wt")
                nc.sync.dma_start(out=wt_sb[:], in_=wt[i])
                yst = ystage.tile([P, nloc], f32, tag="yst")
                # reuse the psA pool slots (phase A tiles are drained by now)
                ps = [
                    psA.tile([P, free], f32, tag="psA", name=f"psC_{i}_{h}")
                    for h in range(nh)
                ]
                for k in range(kd):
                    for h in range(nh):
                        sl = slice(h * free, (h + 1) * free)
                        nc.tensor.matmul(
                            ps[h][:],
                            lhsT=wt_sb[:, k],
                            rhs=xt_sb[:, k, sl],
                            start=(k == 0),
                            stop=(k == kd - 1),
                        )
                for h in range(nh):
                    sl = slice(h * free, (h + 1) * free)
                    # yb = psum + bias (ACT, per-partition bias), then
                    # leaky = max(0.01*yb, yb) in one DVE op
                    nc.scalar.activation(
                        yst[:, sl],
                        ps[h][:],
                        Identity,
                        bias=bias_sb[:, i : i + 1],
                    )
                    nc.vector.scalar_tensor_tensor(
                        out=yst[:, sl],
                        in0=yst[:, sl],
                        scalar=NEG_SLOPE,
                        in1=yst[:, sl],
                        op0=mybir.AluOpType.mult,
                        op1=mybir.AluOpType.max,
                    )
                nc.sync.dma_start(out=yt[i], in_=yst[:])

    nc.compile()
    return nc


def prep_inputs(x, atten_emb, W, b, ncores=NCORES):
    """Host-side shard + pre-tile. Returns (in_maps, dims)."""
    x = np.asarray(x, dtype=np.float32)
    A = np.asarray(atten_emb, dtype=np.float32)
    Wf = np.asarray(W, dtype=np.float32)
    bf = np.asarray(b, dtype=np.float32)

    D = x.shape[-1]
    C = A.shape[0]
    N = int(np.prod(x.shape[:-1]))
    nloc = N // ncores
    kd, ct = D // P, C // P

    xf = np.ascontiguousarray(x.reshape(N, D))

    # at[i, p, k, cc] = A[i*P+cc, k*P+p]   (lhsT for mm1: partition=d, free=c)
    at_t = np.ascontiguousarray(
        A.astype(ml_dtypes.bfloat16).reshape(ct, P, kd, P).transpose(0, 3, 2, 1)
    )
    # abf[j, p, i, dd] = A_bf16[i*P+p, j*P+dd]  (lhsT for mm2: partition=c, free=d)
    a_bf = np.ascontiguousarray(
        A.astype(ml_dtypes.bfloat16).reshape(ct, P, kd, P).transpose(2, 1, 0, 3)
    )
    # wt[i, p, k, cc] = W[i*P+cc, k*P+p]   (lhsT for mm3)
    wt_t = np.ascontiguousarray(Wf.reshape(ct, P, kd, P).transpose(0, 3, 2, 1))
    # bias[p, i] = b[i*P+p]
    bias_t = np.ascontiguousarray(bf.reshape(ct, P).T)

    in_maps = []
    for core in range(ncores):
        xl = xf[core * nloc : (core + 1) * nloc]  # [nloc, D]
        # xt[p, k, n] = xl[n, k*P+p]
        xt = np.ascontiguousarray(xl.T.reshape(kd, P, nloc).transpose(1, 0, 2))
        in_maps.append(
            {
                "xt": xt,
                "xbf": xt.astype(ml_dtypes.bfloat16),
                "at": at_t,
                "abf": a_bf,
                "wt": wt_t,
                "bias": bias_t,
            }
        )
    return in_maps, (N, D, C, nloc)


def assemble_output(results, shape_in, C, nloc):
    """Gather per-core yt [ct, P, nloc] -> full y with original leading dims."""
    N = int(np.prod(shape_in[:-1]))
    y = np.empty((N, C), dtype=np.float32)
    for core, r in enumerate(results):
        yt = r["yt"]  # [ct, P, nloc]; yt[i, p, n] = y_core[n, i*P+p]
        y[core * nloc : (core + 1) * nloc] = yt.reshape(C, nloc).T
    return y.reshape(shape_in[:-1] + (C,))


_CACHED = {}


def _get_nc(nloc, d, c):
    key = (nloc, d, c)
    if key not in _CACHED:
        _CACHED[key] = build_bass(nloc, d, c)
    return _CACHED[key]


def kernel(x, atten_emb, W, b, trace=False):
    from concourse.bass_utils import run_bass_kernel_spmd

    in_maps, (N, D, C, nloc) = prep_inputs(x, atten_emb, W, b)
    nc = _get_nc(nloc, D, C)
    res = run_bass_kernel_spmd(
        nc, in_maps, core_ids=list(range(NCORES)), trace=trace
    )
    y = assemble_output(res.results, tuple(x.shape), C, nloc)
    if trace:
        kernel.last_result = res
    return y
